# revision 20
# baseline (speedup 1.0000x reference)
"""Cross-attention kernel for trn2, 8 NeuronCores.

Problem: x[4,1024,512], context[4,8192,512], Wq[512,512], Wkv[512,1024],
Wout[512,512], bout[512]; 8 heads x 64 dim; out[4,1024,512].

Sharding: core c -> batch b=c//2, head-group g=c%2 (4 heads each).
Each core computes partial_out_b = sum_{h in g} softmax(q_h k_h^T/8) v_h @ Wout_h.
Host: out[b] = partial[2b] + partial[2b+1] + bout.

Schedule notes (engines execute their streams in order, so overlap is
explicit in program order):
  - ctx is processed in 4 groups of 2048 cols. kT/V' of a group are SBUF
    resident; attention runs 4 (it, pair) passes per group with the 128-col
    j-chunks innermost, accumulating U^T in PSUM across the group's 16
    chunks (matmul start/stop), so the DVE does one add per pass per head.
  - kv-projection matmuls for the NEXT group (or the next rep's group 0 and
    its q-projection) are pumped 2-at-a-time into the attention stream so
    the PE never drains while ScalarE works through the exps (~1.04us per
    1024-elem exp vs ~0.85us PE per chunk).
  - attnV for chunk u-1 is emitted after scores+exp for chunk u (lag 1;
    st/p PSUM+SBUF rings bufs>=2) so exp results are ready when PE needs
    them.
  - All matmuls float32r (full rate at N>=256).  PSUM: st [128,2,512]x2 +
    ut [65,2,512]x1 + kv [128,512]x2 = exactly 8 banks.
"""

import itertools

import numpy as np

import concourse.bass as bass
import concourse.mybir as mybir
import concourse.tile as tile
from concourse.vector_clock import ScopedClock

DT = mybir.dt
F32 = DT.float32
F32R = DT.float32r
BF16 = DT.bfloat16
AF = mybir.ActivationFunctionType

B, NQ, NC, D = 4, 1024, 8192, 512
H, HD = 8, 64           # total heads, head dim
HPC = 4                 # heads per core
NPAIR = 2               # head pairs per core
NGC = 2048              # ctx cols per group
NGRP = NC // NGC        # 4 groups
JPG = NGC // 128        # 16 j-chunks per group
NIT = NQ // 512         # 2 i-tiles
PASSES = [(0, 0), (0, 1), (1, 0), (1, 1)]  # (it, pair)

_MAX_WAITS = 1


def _patch_drain():
    def _patched(self, tick_clock, wait_clock):
        nc = self.nc
        drain_inst = nc.sync.drain()
        wait_clock.add_sem_waits(
            drain_inst.ins, ScopedClock({None: tick_clock.global_clock})
        )
        si = drain_inst.ins.sync_info
        if si is not None and si.on_wait and len(si.on_wait) > _MAX_WAITS:
            waits = list(si.on_wait)
            drain_inst.ins.sync_info = mybir.SyncInfo(
                on_wait=waits[:_MAX_WAITS], on_update=list(si.on_update or [])
            )
            for i in range(_MAX_WAITS, len(waits), _MAX_WAITS):
                extra = nc.sync.drain()
                extra.ins.sync_info = mybir.SyncInfo(
                    on_wait=waits[i : i + _MAX_WAITS], on_update=[]
                )
        nc.all_engine_barrier()
        assert self.sems is not None
        popped = nc._tile_sem_poison_stack.pop()
        assert popped is self._sem_poison
        nc.clear_and_free_semaphores(list(self.sems.allocated().values()))
        nc.all_engine_barrier()

    tile.TileContext._drain_and_barrier = _patched


def _split_waits(nc):
    """This container's walrus caps sync waits at 1/instruction; hoist the
    excess onto same-engine nops placed immediately before."""
    for fn in nc.m.functions:
        for bb in fn.blocks:
            out, changed = [], False
            for inst in bb.instructions:
                si = inst.sync_info
                if si is not None and si.on_wait and len(si.on_wait) > _MAX_WAITS:
                    waits = list(si.on_wait)
                    extra, keep = waits[:-_MAX_WAITS], waits[-_MAX_WAITS:]
                    for i in range(0, len(extra), _MAX_WAITS):
                        nop = mybir.InstNoOp(
                            name=nc.get_next_instruction_name(),
                            engine=inst.engine,
                            sync_info=mybir.SyncInfo(
                                on_wait=extra[i : i + _MAX_WAITS], on_update=[]
                            ),
                        )
                        nc.register_instruction(nop)
                        out.append(nop)
                    inst.sync_info = mybir.SyncInfo(
                        on_wait=keep, on_update=list(si.on_update or [])
                    )
                    changed = True
                out.append(inst)
            if changed:
                bb.instructions = out


def build_program(reps=1):
    _patch_drain()
    nc = bass.Bass()

    xT = nc.dram_tensor("xT", [D, NQ], F32R, kind="ExternalInput")
    ctxT = nc.dram_tensor("ctxT", [D, NC], F32R, kind="ExternalInput")
    wq = nc.dram_tensor("wq", [D, 256], F32R, kind="ExternalInput")
    wk = nc.dram_tensor("wk", [D, 256], F32R, kind="ExternalInput")
    wv = nc.dram_tensor("wv", [D, 256], F32R, kind="ExternalInput")
    wout = nc.dram_tensor("wout", [256, D], F32R, kind="ExternalInput")
    ones = nc.dram_tensor("ones", [128, 64], F32R, kind="ExternalInput")
    out = nc.dram_tensor("out", [NQ, D], F32, kind="ExternalOutput")

    with tile.TileContext(nc) as tc:
        with (
            tc.tile_pool(name="wp", bufs=1) as wp,
            tc.tile_pool(name="qt", bufs=2) as qtp,
            tc.tile_pool(name="ctx", bufs=2) as ctxp,
            tc.tile_pool(name="kt", bufs=2) as ktp,
            tc.tile_pool(name="vt", bufs=2) as vtp,
            tc.tile_pool(name="vb", bufs=2) as vbp,
            tc.tile_pool(name="pp", bufs=3) as ppp,
            tc.tile_pool(name="uts", bufs=1) as utsp,
            tc.tile_pool(name="outp", bufs=2) as outp,
            tc.tile_pool(name="eps", bufs=8) as epsp,
            tc.tile_pool(name="ps", bufs=1, space="PSUM") as psp,
        ):
            # ---- load weights / xT (once per program) ----
            wq_sb = wp.tile([128, 4, 256], F32R, tag="wq")
            wk_sb = wp.tile([128, 4, 256], F32R, tag="wk")
            wv_sb = wp.tile([128, 4, 256], F32R, tag="wv")
            wout_sb = wp.tile([64, 4, D], F32R, tag="wout")
            xT_sb = wp.tile([128, 4, NQ], F32R, tag="xT")
            nc.sync.dma_start(out=wq_sb, in_=wq.rearrange("(c p) m -> p c m", p=128))
            nc.sync.dma_start(out=wk_sb, in_=wk.rearrange("(c p) m -> p c m", p=128))
            nc.sync.dma_start(out=wv_sb, in_=wv.rearrange("(c p) m -> p c m", p=128))
            nc.sync.dma_start(
                out=wout_sb, in_=wout.rearrange("(h p) n -> p h n", p=64)
            )
            nc.sync.dma_start(out=xT_sb, in_=xT.rearrange("(c p) n -> p c n", p=128))
            ones_sb = wp.tile([128, 64], F32R, tag="ones")
            nc.sync.dma_start(out=ones_sb, in_=ones[:, :])

            def st_tile():
                return psp.tile([128, 2, 512], F32, tag="st", bufs=2, name="st")

            def ut_tile():
                return psp.tile([65, 2, 512], F32, tag="ut", bufs=1, name="utps")

            def kv_tile():
                return psp.tile([128, 512], F32, tag="kv", bufs=2, name="kvps")

            def emit_q_proj():
                """qT[pair][128, NQ] = Wq^T x^T; 16 matmuls, 4 copies."""
                qT = [
                    qtp.tile([128, NQ], F32R, tag=f"qt{p}", name=f"qT{p}")
                    for p in range(NPAIR)
                ]
                for pair in range(NPAIR):
                    for it in range(NIT):
                        qps = kv_tile()
                        for kc in range(4):
                            nc.tensor.matmul(
                                qps,
                                wq_sb[:, kc, pair * 128 : (pair + 1) * 128],
                                xT_sb[:, kc, it * 512 : (it + 1) * 512],
                                start=(kc == 0),
                                stop=(kc == 3),
                            )
                            yield None
                        nc.vector.tensor_copy(
                            out=qT[pair][:, it * 512 : (it + 1) * 512], in_=qps
                        )
                emit_q_proj.result = qT

            def emit_kv_proj(g):
                """kT_g [128, 2, NGC], v_g [128, JPG, HPC, 65] for ctx group g.
                Yields after every matmul so the caller can interleave.
                Half-0 matmuls come first so the ctx DMAs stay ahead."""
                base = g * NGC
                ctx_sb = [
                    ctxp.tile([128, 4, 1024], F32R, tag="ctx", name=f"ctx{g}_{i}")
                    for i in range(2)
                ]
                for half in range(2):
                    for kc in range(4):
                        nc.sync.dma_start(
                            out=ctx_sb[half][:, kc, :],
                            in_=ctxT[
                                kc * 128 : (kc + 1) * 128,
                                base + half * 1024 : base + (half + 1) * 1024,
                            ],
                        )
                kT_g = ktp.tile([128, NPAIR, NGC], F32R, tag="kt", name=f"kT{g}")
                v_g = vbp.tile([128, JPG, HPC, 65], BF16, tag="vb", name=f"v{g}")
                vT_sb = [
                    vtp.tile([128, NGC], BF16, tag=f"vt{dh}", name=f"vT{g}_{dh}")
                    for dh in range(2)
                ]
                nc.vector.tensor_copy(
                    out=v_g[:, :, :, 64:65],
                    in_=ones_sb.rearrange("p (j h o) -> p j h o", j=JPG, h=HPC),
                )
                # k-projection: half-major so half 1's DMA can land
                for half in range(2):
                    for pair in range(NPAIR):
                        for sub in range(2):
                            nt = half * 2 + sub
                            kps = kv_tile()
                            for kc in range(4):
                                nc.tensor.matmul(
                                    kps,
                                    wk_sb[:, kc, pair * 128 : (pair + 1) * 128],
                                    ctx_sb[half][:, kc, sub * 512 : (sub + 1) * 512],
                                    start=(kc == 0),
                                    stop=(kc == 3),
                                )
                                yield None
                            nc.vector.tensor_copy(
                                out=kT_g[:, pair, nt * 512 : (nt + 1) * 512],
                                in_=kps,
                            )
                # v-projection into d-major vT, then transpose-DMA into the
                # j-major layout the attnV stationary needs.
                for dh in range(2):
                    for half in range(2):
                        for sub in range(2):
                            nt = half * 2 + sub
                            vps = kv_tile()
                            for kc in range(4):
                                nc.tensor.matmul(
                                    vps,
                                    wv_sb[:, kc, dh * 128 : (dh + 1) * 128],
                                    ctx_sb[half][:, kc, sub * 512 : (sub + 1) * 512],
                                    start=(kc == 0),
                                    stop=(kc == 3),
                                )
                                yield None
                            nc.vector.tensor_copy(
                                out=vT_sb[dh][:, nt * 512 : (nt + 1) * 512],
                                in_=vps,
                            )
                    for hsub in range(2):
                        h = dh * 2 + hsub
                        nc.sync.dma_start_transpose(
                            out=v_g[:, :, h, 0:64],
                            in_=vT_sb[dh][hsub * 64 : (hsub + 1) * 64, :],
                        )
                emit_kv_proj.result = (kT_g, v_g)

            def emit_scale(ut_sb, pair):
                """Scale pair's heads by recip(colsum) in place: the later
                out-projection then sums heads in PSUM with no DVE coupling."""
                for hh in range(2):
                    h = pair * 2 + hh
                    with nc.allow_low_precision(reason="f32r is fp32 storage"):
                        nc.vector.reciprocal(
                            out=ut_sb[h][64:65, :], in_=ut_sb[h][64:65, :]
                        )
                    for it in range(NIT):
                        rbps = kv_tile()
                        # K=1 matmul: ones column x recip row -> [64, 512]
                        nc.tensor.matmul(
                            rbps[0:64, :],
                            ones_sb[64:65, 0:64],
                            ut_sb[h][64:65, it * 512 : (it + 1) * 512],
                            start=True,
                            stop=True,
                        )
                        seg = ut_sb[h][0:64, it * 512 : (it + 1) * 512]
                        nc.vector.tensor_mul(
                            out=seg, in0=seg, in1=rbps[0:64, :]
                        )

            def emit_epilogue(ut_sb):
                """out-projection on pre-scaled U, heads summed in PSUM,
                stored straight from PSUM."""
                for it in range(NIT):
                    for ic in range(4):
                        ops = st_tile()[:, 0, :]
                        for h in range(HPC):
                            nc.tensor.matmul(
                                ops,
                                ut_sb[h][
                                    0:64,
                                    it * 512 + ic * 128 : it * 512 + (ic + 1) * 128,
                                ],
                                wout_sb[:, h, :],
                                start=(h == 0),
                                stop=(h == HPC - 1),
                            )
                        acc = outp.tile([128, 512], F32, tag="outp", name="acc")
                        nc.vector.tensor_copy(out=acc, in_=ops)
                        nc.sync.dma_start(
                            out=out[
                                it * 512 + ic * 128 : it * 512 + (ic + 1) * 128, :
                            ],
                            in_=acc,
                        )

            def run_gen(gen):
                if gen is not None:
                    for _ in gen:
                        pass

            def pump(gen, n):
                if gen is None:
                    return None
                for _ in range(n):
                    try:
                        next(gen)
                    except StopIteration:
                        return None
                return gen

            def drain_pass(ut_ps, it, pair, first_group):
                for hh in range(2):
                    h = pair * 2 + hh
                    seg = ut_sb[h][:, it * 512 : (it + 1) * 512]
                    if first_group:
                        nc.vector.tensor_copy(out=seg, in_=ut_ps[:, hh, :])
                    else:
                        nc.vector.tensor_add(out=seg, in0=seg, in1=ut_ps[:, hh, :])

            # ---- prologue: q-proj + kv group 0 of rep 0, un-overlapped ----
            run_gen(emit_q_proj())
            qT = emit_q_proj.result
            run_gen(emit_kv_proj(0))
            kv_cur = emit_kv_proj.result

            for _rep in range(reps):
                ut_sb = [
                    utsp.tile([65, NQ], F32R, tag=f"uts{h}", name=f"ut{h}")
                    for h in range(HPC)
                ]
                for g in range(NGRP):
                    kT_g, v_g = kv_cur
                    has_next = (g < NGRP - 1) or (_rep < reps - 1)
                    if g < NGRP - 1:
                        side = emit_kv_proj(g + 1)
                    elif _rep < reps - 1:
                        side = itertools.chain(emit_kv_proj(0), emit_q_proj())
                    else:
                        side = None

                    prev = None  # (ut_ps, it, pair, jj, p_sb)
                    for ip, (it, pair) in enumerate(PASSES):
                        ut_ps = ut_tile()
                        for jj in range(JPG):
                            unit = ip * JPG + jj
                            st = st_tile()
                            for hh in range(2):
                                b0 = hh * 64
                                nc.tensor.matmul(
                                    st[:, hh, :],
                                    kT_g[
                                        b0 : b0 + 64,
                                        pair,
                                        jj * 128 : (jj + 1) * 128,
                                    ],
                                    qT[pair][
                                        b0 : b0 + 64, it * 512 : (it + 1) * 512
                                    ],
                                    start=True,
                                    stop=True,
                                )
                            p_sb = ppp.tile([128, 2, 512], BF16, tag="pp", name="p")
                            nc.scalar.activation(
                                out=p_sb, in_=st, func=AF.Exp, scale=0.125
                            )
                            if unit >= 4:
                                side = pump(side, 1)
                            if prev is not None:
                                p_ut, p_it, p_pair, p_jj, p_p = prev
                                for hh in range(2):
                                    h = p_pair * 2 + hh
                                    nc.tensor.matmul(
                                        p_ut[:, hh, :],
                                        v_g[:, p_jj, h, :],
                                        p_p[:, hh, :],
                                        start=(p_jj == 0),
                                        stop=(p_jj == JPG - 1),
                                    )
                                if p_jj == JPG - 1:
                                    drain_pass(p_ut, p_it, p_pair, g == 0)
                                    if g == NGRP - 1 and p_it == 1:
                                        emit_scale(ut_sb, p_pair)
                            prev = (ut_ps, it, pair, jj, p_sb)
                    # flush the group's last chunk
                    p_ut, p_it, p_pair, p_jj, p_p = prev
                    for hh in range(2):
                        h = p_pair * 2 + hh
                        nc.tensor.matmul(
                            p_ut[:, hh, :],
                            v_g[:, p_jj, h, :],
                            p_p[:, hh, :],
                            start=(p_jj == 0),
                            stop=True,
                        )
                    drain_pass(p_ut, p_it, p_pair, g == 0)
                    if g == NGRP - 1:
                        emit_scale(ut_sb, p_pair)
                    run_gen(side)
                    if has_next:
                        kv_cur = emit_kv_proj.result
                    if g == NGRP - 1 and _rep < reps - 1:
                        qT = emit_q_proj.result

                emit_epilogue(ut_sb)

    _split_waits(nc)
    return nc


_NC_CACHE = None


def _get_program():
    global _NC_CACHE
    if _NC_CACHE is None:
        _NC_CACHE = build_program()
    return _NC_CACHE


def make_in_maps(x, context, Wq, Wkv, Wout):
    """Host-side shard + layout prep: slice per (batch, head-group), transpose
    activations to feature-major."""
    f32 = np.float32
    in_maps = []
    Wk = Wkv[:, : H * HD]
    Wv = Wkv[:, H * HD :]
    for c in range(8):
        b, g = c // 2, c % 2
        hs = g * HPC * HD  # 256*g
        in_maps.append(
            {
                "xT": np.ascontiguousarray(x[b].T.astype(f32)),
                "ctxT": np.ascontiguousarray(context[b].T.astype(f32)),
                "wq": np.ascontiguousarray(Wq[:, hs : hs + 256].astype(f32)),
                "wk": np.ascontiguousarray(Wk[:, hs : hs + 256].astype(f32)),
                "wv": np.ascontiguousarray(Wv[:, hs : hs + 256].astype(f32)),
                "wout": np.ascontiguousarray(Wout[hs : hs + 256, :].astype(f32)),
                "ones": np.ones((128, 64), dtype=f32),
            }
        )
    return in_maps


def kernel(x, context, Wq, Wkv, Wout, bout):
    from concourse.bass_utils import run_bass_kernel_spmd

    nc = _get_program()
    in_maps = make_in_maps(x, context, Wq, Wkv, Wout)
    res = run_bass_kernel_spmd(nc, in_maps, core_ids=list(range(8)))
    outs = [res.results[c]["out"] for c in range(8)]
    full = np.empty((B, NQ, D), dtype=np.float32)
    for b in range(B):
        full[b] = outs[2 * b] + outs[2 * b + 1] + bout.astype(np.float32)
    return full


# revision 25
# speedup vs baseline: 1.2549x; 1.2549x over previous
"""Cross-attention kernel for trn2, 8 NeuronCores.

Problem: x[4,1024,512], context[4,8192,512], Wq[512,512], Wkv[512,1024],
Wout[512,512], bout[512]; 8 heads x 64 dim; out[4,1024,512].

Sharding: core c -> batch b=c//2, head-group g=c%2 (4 heads each).
Each core computes partial_out_b = sum_{h in g} softmax(q_h k_h^T/8) v_h @ Wout_h.
Host: out[b] = partial[2b] + partial[2b+1] + bout.

Schedule notes (engines execute their streams in order, so overlap is
explicit in program order):
  - ctx is processed in 4 groups of 2048 cols. kT/V' of a group are SBUF
    resident; attention runs 4 (it, pair) passes per group with the 128-col
    j-chunks innermost, accumulating U^T in PSUM across the group's 16
    chunks (matmul start/stop), so the DVE does one add per pass per head.
  - kv-projection matmuls for the NEXT group (or the next rep's group 0 and
    its q-projection) are pumped 2-at-a-time into the attention stream so
    the PE never drains while ScalarE works through the exps (~1.04us per
    1024-elem exp vs ~0.85us PE per chunk).
  - attnV for chunk u-1 is emitted after scores+exp for chunk u (lag 1;
    st/p PSUM+SBUF rings bufs>=2) so exp results are ready when PE needs
    them.
  - All matmuls float32r (full rate at N>=256).  PSUM: st [128,2,512]x2 +
    ut [65,2,512]x1 + kv [128,512]x2 = exactly 8 banks.
"""

import itertools

import numpy as np

import concourse.bass as bass
import concourse.mybir as mybir
import concourse.tile as tile
from concourse.vector_clock import ScopedClock

DT = mybir.dt
F32 = DT.float32
F32R = DT.float32r
AF = mybir.ActivationFunctionType

B, NQ, NC, D = 4, 1024, 8192, 512
H, HD = 8, 64           # total heads, head dim
HPC = 4                 # heads per core
NPAIR = 2               # head pairs per core
NGC = 2048              # ctx cols per group
NGRP = NC // NGC        # 4 groups
JPG = NGC // 128        # 16 j-chunks per group
NIT = NQ // 512         # 2 i-tiles
PASSES = [(0, 0), (0, 1), (1, 0), (1, 1)]  # (it, pair)

_MAX_WAITS = 1


def _patch_drain():
    def _patched(self, tick_clock, wait_clock):
        nc = self.nc
        drain_inst = nc.sync.drain()
        wait_clock.add_sem_waits(
            drain_inst.ins, ScopedClock({None: tick_clock.global_clock})
        )
        si = drain_inst.ins.sync_info
        if si is not None and si.on_wait and len(si.on_wait) > _MAX_WAITS:
            waits = list(si.on_wait)
            drain_inst.ins.sync_info = mybir.SyncInfo(
                on_wait=waits[:_MAX_WAITS], on_update=list(si.on_update or [])
            )
            for i in range(_MAX_WAITS, len(waits), _MAX_WAITS):
                extra = nc.sync.drain()
                extra.ins.sync_info = mybir.SyncInfo(
                    on_wait=waits[i : i + _MAX_WAITS], on_update=[]
                )
        nc.all_engine_barrier()
        assert self.sems is not None
        popped = nc._tile_sem_poison_stack.pop()
        assert popped is self._sem_poison
        nc.clear_and_free_semaphores(list(self.sems.allocated().values()))
        nc.all_engine_barrier()

    tile.TileContext._drain_and_barrier = _patched


def _split_waits(nc):
    """This container's walrus caps sync waits at 1/instruction; hoist the
    excess onto same-engine nops placed immediately before."""
    for fn in nc.m.functions:
        for bb in fn.blocks:
            out, changed = [], False
            for inst in bb.instructions:
                si = inst.sync_info
                if si is not None and si.on_wait and len(si.on_wait) > _MAX_WAITS:
                    waits = list(si.on_wait)
                    extra, keep = waits[:-_MAX_WAITS], waits[-_MAX_WAITS:]
                    for i in range(0, len(extra), _MAX_WAITS):
                        nop = mybir.InstNoOp(
                            name=nc.get_next_instruction_name(),
                            engine=inst.engine,
                            sync_info=mybir.SyncInfo(
                                on_wait=extra[i : i + _MAX_WAITS], on_update=[]
                            ),
                        )
                        nc.register_instruction(nop)
                        out.append(nop)
                    inst.sync_info = mybir.SyncInfo(
                        on_wait=keep, on_update=list(si.on_update or [])
                    )
                    changed = True
                out.append(inst)
            if changed:
                bb.instructions = out


def build_program(reps=1):
    _patch_drain()
    nc = bass.Bass()

    xT = nc.dram_tensor("xT", [D, NQ], F32R, kind="ExternalInput")
    ctxT = nc.dram_tensor("ctxT", [D, NC], F32R, kind="ExternalInput")
    wq = nc.dram_tensor("wq", [D, 256], F32R, kind="ExternalInput")
    wk = nc.dram_tensor("wk", [D, 256], F32R, kind="ExternalInput")
    wv = nc.dram_tensor("wv", [D, 256], F32R, kind="ExternalInput")
    wout = nc.dram_tensor("wout", [256, D], F32R, kind="ExternalInput")
    ones = nc.dram_tensor("ones", [128, 64], F32R, kind="ExternalInput")
    out = nc.dram_tensor("out", [NQ, D], F32, kind="ExternalOutput")

    with tile.TileContext(nc) as tc:
        with (
            tc.tile_pool(name="wp", bufs=1) as wp,
            tc.tile_pool(name="qt", bufs=2) as qtp,
            tc.tile_pool(name="ctx", bufs=2) as ctxp,
            tc.tile_pool(name="kt", bufs=2) as ktp,
            tc.tile_pool(name="vb", bufs=2) as vbp,
            tc.tile_pool(name="pp", bufs=3) as ppp,
            tc.tile_pool(name="uts", bufs=1) as utsp,
            tc.tile_pool(name="outp", bufs=2) as outp,
            tc.tile_pool(name="eps", bufs=8) as epsp,
            tc.tile_pool(name="ps", bufs=1, space="PSUM") as psp,
        ):
            # ---- load weights / xT (once per program) ----
            wq_sb = wp.tile([128, 4, 256], F32R, tag="wq")
            wk_sb = wp.tile([128, 4, 256], F32R, tag="wk")
            wv_sb = wp.tile([128, 4, 256], F32R, tag="wv")
            wout_sb = wp.tile([64, 4, D], F32R, tag="wout")
            xT_sb = wp.tile([128, 4, NQ], F32R, tag="xT")
            nc.sync.dma_start(out=wq_sb, in_=wq.rearrange("(c p) m -> p c m", p=128))
            nc.sync.dma_start(out=wk_sb, in_=wk.rearrange("(c p) m -> p c m", p=128))
            nc.sync.dma_start(out=wv_sb, in_=wv.rearrange("(c p) m -> p c m", p=128))
            nc.sync.dma_start(
                out=wout_sb, in_=wout.rearrange("(h p) n -> p h n", p=64)
            )
            nc.sync.dma_start(out=xT_sb, in_=xT.rearrange("(c p) n -> p c n", p=128))
            ones_sb = wp.tile([128, 64], F32R, tag="ones")
            nc.sync.dma_start(out=ones_sb, in_=ones[:, :])

            def st_tile():
                return psp.tile([128, 2, 512], F32, tag="st", bufs=2, name="st")

            def ut_tile():
                return psp.tile([65, 2, 512], F32, tag="ut", bufs=1, name="utps")

            def kv_tile():
                return psp.tile([128, 512], F32, tag="kv", bufs=2, name="kvps")

            def emit_q_proj():
                """qT[pair][128, NQ] = Wq^T x^T; 16 matmuls, 4 copies."""
                qT = [
                    qtp.tile([128, NQ], F32R, tag=f"qt{p}", name=f"qT{p}")
                    for p in range(NPAIR)
                ]
                for pair in range(NPAIR):
                    for it in range(NIT):
                        qps = kv_tile()
                        for kc in range(4):
                            nc.tensor.matmul(
                                qps,
                                wq_sb[:, kc, pair * 128 : (pair + 1) * 128],
                                xT_sb[:, kc, it * 512 : (it + 1) * 512],
                                start=(kc == 0),
                                stop=(kc == 3),
                            )
                            yield None
                        nc.vector.tensor_copy(
                            out=qT[pair][:, it * 512 : (it + 1) * 512], in_=qps
                        )
                emit_q_proj.result = qT

            def emit_kv_proj(g):
                """kT_g [128, 2, NGC], v_g [128, JPG, HPC, 65] for ctx group g.
                Yields after every matmul so the caller can interleave.
                Half-0 matmuls come first so the ctx DMAs stay ahead."""
                base = g * NGC
                ctx_sb = [
                    ctxp.tile([128, 4, 1024], F32R, tag="ctx", name=f"ctx{g}_{i}")
                    for i in range(2)
                ]
                for half in range(2):
                    for kc in range(4):
                        nc.sync.dma_start(
                            out=ctx_sb[half][:, kc, :],
                            in_=ctxT[
                                kc * 128 : (kc + 1) * 128,
                                base + half * 1024 : base + (half + 1) * 1024,
                            ],
                        )
                kT_g = ktp.tile([128, NPAIR, NGC], F32R, tag="kt", name=f"kT{g}")
                v_g = vbp.tile([128, JPG, HPC, 65], F32R, tag="vb", name=f"v{g}")
                nc.vector.tensor_copy(
                    out=v_g[:, :, :, 64:65],
                    in_=ones_sb.rearrange("p (j h o) -> p j h o", j=JPG, h=HPC),
                )
                # k-projection: half-major so half 1's DMA has time to land
                for half in range(2):
                    for pair in range(NPAIR):
                        for sub in range(2):
                            nt = half * 2 + sub
                            kps = kv_tile()
                            for kc in range(4):
                                nc.tensor.matmul(
                                    kps,
                                    wk_sb[:, kc, pair * 128 : (pair + 1) * 128],
                                    ctx_sb[half][:, kc, sub * 512 : (sub + 1) * 512],
                                    start=(kc == 0),
                                    stop=(kc == 3),
                                )
                                yield None
                            nc.vector.tensor_copy(
                                out=kT_g[:, pair, nt * 512 : (nt + 1) * 512],
                                in_=kps,
                            )
                # v-projection: 16 jj x 4 kc
                for jj in range(JPG):
                    half, joff = jj // 8, (jj % 8) * 128
                    vps = kv_tile()
                    for kc in range(4):
                        nc.tensor.matmul(
                            vps[:, 0:256],
                            ctx_sb[half][:, kc, joff : joff + 128],
                            wv_sb[:, kc, :],
                            start=(kc == 0),
                            stop=(kc == 3),
                        )
                        yield None
                    nc.vector.tensor_copy(
                        out=v_g[:, jj, :, 0:64],
                        in_=vps[:, 0:256].rearrange("p (h x) -> p h x", h=HPC),
                    )
                emit_kv_proj.result = (kT_g, v_g)

            def emit_epilogue(ut_sb):
                """recip(colsum), out-projection, scale+sum heads, store."""
                for it in range(NIT):
                    recips = []
                    for h in range(HPC):
                        cs_t = epsp.tile([128, 4], F32R, tag="cs", name="cs")
                        for ic in range(4):
                            nc.sync.dma_start(
                                out=cs_t[:, ic : ic + 1],
                                in_=ut_sb[h][
                                    64:65,
                                    it * 512 + ic * 128 : it * 512 + (ic + 1) * 128,
                                ],
                            )
                        rec = epsp.tile([128, 4], F32, tag="rec", name="rec")
                        nc.vector.reciprocal(out=rec, in_=cs_t)
                        recips.append(rec)
                    acc = outp.tile([128, 4, 512], F32, tag="outp", name="acc")
                    for ic in range(4):
                        for h in range(HPC):
                            ops = kv_tile()
                            nc.tensor.matmul(
                                ops,
                                ut_sb[h][
                                    0:64,
                                    it * 512 + ic * 128 : it * 512 + (ic + 1) * 128,
                                ],
                                wout_sb[:, h, :],
                                start=True,
                                stop=True,
                            )
                            if h == 0:
                                nc.vector.tensor_scalar_mul(
                                    out=acc[:, ic, :],
                                    in0=ops,
                                    scalar1=recips[h][:, ic : ic + 1],
                                )
                            else:
                                nc.vector.scalar_tensor_tensor(
                                    out=acc[:, ic, :],
                                    in0=ops,
                                    scalar=recips[h][:, ic : ic + 1],
                                    in1=acc[:, ic, :],
                                    op0=mybir.AluOpType.mult,
                                    op1=mybir.AluOpType.add,
                                )
                    nc.sync.dma_start(
                        out=out[it * 512 : (it + 1) * 512, :].rearrange(
                            "(c p) n -> p c n", p=128
                        ),
                        in_=acc,
                    )

            def run_gen(gen):
                if gen is not None:
                    for _ in gen:
                        pass

            def pump(gen, n):
                if gen is None:
                    return None
                for _ in range(n):
                    try:
                        next(gen)
                    except StopIteration:
                        return None
                return gen

            def drain_pass(ut_ps, it, pair, first_group):
                for hh in range(2):
                    h = pair * 2 + hh
                    seg = ut_sb[h][:, it * 512 : (it + 1) * 512]
                    if first_group:
                        nc.vector.tensor_copy(out=seg, in_=ut_ps[:, hh, :])
                    else:
                        nc.vector.tensor_add(out=seg, in0=seg, in1=ut_ps[:, hh, :])

            # ---- prologue: q-proj + kv group 0 of rep 0, un-overlapped ----
            run_gen(emit_q_proj())
            qT = emit_q_proj.result
            run_gen(emit_kv_proj(0))
            kv_cur = emit_kv_proj.result

            for _rep in range(reps):
                ut_sb = [
                    utsp.tile([65, NQ], F32R, tag=f"uts{h}", name=f"ut{h}")
                    for h in range(HPC)
                ]
                for g in range(NGRP):
                    kT_g, v_g = kv_cur
                    has_next = (g < NGRP - 1) or (_rep < reps - 1)
                    if g < NGRP - 1:
                        side = emit_kv_proj(g + 1)
                    elif _rep < reps - 1:
                        side = itertools.chain(emit_kv_proj(0), emit_q_proj())
                    else:
                        side = None

                    prev = None  # (ut_ps, it, pair, jj, p_sb)
                    for ip, (it, pair) in enumerate(PASSES):
                        ut_ps = ut_tile()
                        for jj in range(JPG):
                            unit = ip * JPG + jj
                            st = st_tile()
                            for hh in range(2):
                                b0 = hh * 64
                                nc.tensor.matmul(
                                    st[:, hh, :],
                                    kT_g[
                                        b0 : b0 + 64,
                                        pair,
                                        jj * 128 : (jj + 1) * 128,
                                    ],
                                    qT[pair][
                                        b0 : b0 + 64, it * 512 : (it + 1) * 512
                                    ],
                                    start=True,
                                    stop=True,
                                )
                            p_sb = ppp.tile([128, 2, 512], F32R, tag="pp", name="p")
                            nc.scalar.activation(
                                out=p_sb, in_=st, func=AF.Exp, scale=0.125
                            )
                            if unit >= 4:
                                side = pump(side, 2)
                            if prev is not None:
                                p_ut, p_it, p_pair, p_jj, p_p = prev
                                for hh in range(2):
                                    h = p_pair * 2 + hh
                                    nc.tensor.matmul(
                                        p_ut[:, hh, :],
                                        v_g[:, p_jj, h, :],
                                        p_p[:, hh, :],
                                        start=(p_jj == 0),
                                        stop=(p_jj == JPG - 1),
                                    )
                                if p_jj == JPG - 1:
                                    drain_pass(p_ut, p_it, p_pair, g == 0)
                            prev = (ut_ps, it, pair, jj, p_sb)
                    # flush the group's last chunk
                    p_ut, p_it, p_pair, p_jj, p_p = prev
                    for hh in range(2):
                        h = p_pair * 2 + hh
                        nc.tensor.matmul(
                            p_ut[:, hh, :],
                            v_g[:, p_jj, h, :],
                            p_p[:, hh, :],
                            start=(p_jj == 0),
                            stop=True,
                        )
                    drain_pass(p_ut, p_it, p_pair, g == 0)
                    run_gen(side)
                    if has_next:
                        kv_cur = emit_kv_proj.result
                    if g == NGRP - 1 and _rep < reps - 1:
                        qT = emit_q_proj.result

                emit_epilogue(ut_sb)

    _split_waits(nc)
    return nc


_NC_CACHE = None


def _get_program():
    global _NC_CACHE
    if _NC_CACHE is None:
        _NC_CACHE = build_program()
    return _NC_CACHE


def make_in_maps(x, context, Wq, Wkv, Wout):
    """Host-side shard + layout prep: slice per (batch, head-group), transpose
    activations to feature-major."""
    f32 = np.float32
    in_maps = []
    Wk = Wkv[:, : H * HD]
    Wv = Wkv[:, H * HD :]
    for c in range(8):
        b, g = c // 2, c % 2
        hs = g * HPC * HD  # 256*g
        in_maps.append(
            {
                "xT": np.ascontiguousarray(x[b].T.astype(f32)),
                "ctxT": np.ascontiguousarray(context[b].T.astype(f32)),
                "wq": np.ascontiguousarray(Wq[:, hs : hs + 256].astype(f32)),
                "wk": np.ascontiguousarray(Wk[:, hs : hs + 256].astype(f32)),
                "wv": np.ascontiguousarray(Wv[:, hs : hs + 256].astype(f32)),
                "wout": np.ascontiguousarray(Wout[hs : hs + 256, :].astype(f32)),
                "ones": np.ones((128, 64), dtype=f32),
            }
        )
    return in_maps


def kernel(x, context, Wq, Wkv, Wout, bout):
    from concourse.bass_utils import run_bass_kernel_spmd

    nc = _get_program()
    in_maps = make_in_maps(x, context, Wq, Wkv, Wout)
    res = run_bass_kernel_spmd(nc, in_maps, core_ids=list(range(8)))
    outs = [res.results[c]["out"] for c in range(8)]
    full = np.empty((B, NQ, D), dtype=np.float32)
    for b in range(B):
        full[b] = outs[2 * b] + outs[2 * b + 1] + bout.astype(np.float32)
    return full
